# revision 1
# baseline (speedup 1.0000x reference)
"""Quantized (4-bit) LoRA linear for Trainium2, SPMD over 8 NeuronCores.

Math:  y[t,o] = sum_i x[t,i]*W[o,i] + bias[o] + 2.0 * sum_r (x@A^T)[t,r]*B[o,r]
where  W[o,i] = (nib[o,i] - zero[i]) * scale[i],  nib = unpacked 4-bit ints.

Rewrite with xs[t,i] = x[t,i]*scale[i]:
  y[t,o] = sum_i xs[t,i]*nib[o,i]        (PE matmul: fp16 xs x fp8 nib, both exact
                                          enough: nib in {0..15} is exact in fp8e4)
         + G[t,:] @ H[o,:]^T             (K=9 matmul folding LoRA + zero-correction)
         + bias[o]                       (fused into the DVE PSUM evacuation)
  G cols: 0-7 = u[t,r] = sum_i xs[t,i]*(A[r,i]/scale[i]) = (x @ A^T)[t,r],
          8   = c[t]   = sum_i xs[t,i]*zero[i]
  H rows: 0-7 = 2.0*B^T, 8 = -1

Sharding: 8-way token split (1024 tokens per core), each core computes the
full 4096 outs in two o-half passes. xs (64 KB/part) stays resident in SBUF
across both passes; the fp8 nib half (64 KB/part) streams through a shared
pool, reloaded for pass 1 pipelined behind pass 0's tail. u/G computed once
(pass 0) and reused in pass 1. All accumulation in PSUM (7+1 banks).
"""

import numpy as np

B, S, I, O = 4, 2048, 4096, 4096
T = B * S            # 8192 tokens
NCORES = 8
TC = T // NCORES     # 1024 tokens per core
OH = O // 2          # 2048 outs per pass
KC = I // 128        # 32 contraction chunks
TG = 4               # token tiles per token-group (512 tokens)
NG = TC // (TG * 128)  # 2 token groups per core

_CACHE = {}


def _build_program():
    import concourse.bacc as bacc
    import concourse.mybir as mybir
    import concourse.tile as tile

    fp16 = mybir.dt.float16
    fp32 = mybir.dt.float32
    fp8 = mybir.dt.float8e4

    nc = bacc.Bacc("TRN2", target_bir_lowering=False, debug=False)
    xsT = nc.dram_tensor("xsT", [I, TC], fp16, kind="ExternalInput")
    nibT = nc.dram_tensor("nibT", [I, O], fp8, kind="ExternalInput")
    aextT = nc.dram_tensor("aextT", [I, 9], fp16, kind="ExternalInput")
    hmat = nc.dram_tensor("hmat", [9, O], fp16, kind="ExternalInput")
    bias_bc = nc.dram_tensor("bias_bc", [128, O], fp32, kind="ExternalInput")
    y = nc.dram_tensor("y", [TC, O], fp32, kind="ExternalOutput")

    with tile.TileContext(nc) as tc:
        with (
            tc.tile_pool(name="nib", bufs=KC) as nib_pool,
            tc.tile_pool(name="consts", bufs=1) as const_pool,
            tc.tile_pool(name="xs", bufs=1) as xs_pool,
            tc.tile_pool(name="g", bufs=1) as g_pool,
            tc.tile_pool(name="out", bufs=3) as out_pool,
            tc.tile_pool(name="psum", bufs=8, space="PSUM") as psum_pool,
        ):
            h_tile = const_pool.tile([9, O], fp16, tag="h")
            bias_tile = const_pool.tile([128, O], fp32, tag="bias")
            aext_tiles = [None] * KC
            # xs resident across both passes: unique tag per tile, bufs=1
            xs_tiles = {}   # (tg, k) -> tile
            gts = [None] * NG

            for h in range(2):  # o-half pass
                o0 = h * OH
                nib_tiles = [None] * KC
                for tg in range(NG):
                    t0 = tg * TG * 128
                    # Interleave DMAs chunk-by-chunk with compute emission so
                    # the PE streams right behind the DMA (pass 0 tg 0 and the
                    # pass-1 nib reload both pipeline this way).
                    for k in range(KC):
                        if tg == 0:
                            nt = nib_pool.tile([128, OH], fp8, tag="nib",
                                               name=f"nib{h}_{k}")
                            nc.sync.dma_start(
                                nt[:], nibT[k * 128:(k + 1) * 128, o0:o0 + OH]
                            )
                            nib_tiles[k] = nt
                        if h == 0:
                            xt = xs_pool.tile([128, TG * 128], fp16,
                                              tag=f"xs{tg}_{k}", bufs=1,
                                              name=f"xs{tg}_{k}")
                            nc.sync.dma_start(
                                xt[:], xsT[k * 128:(k + 1) * 128,
                                           t0:t0 + TG * 128]
                            )
                            xs_tiles[(tg, k)] = xt
                            if tg == 0:
                                at = const_pool.tile([128, 9], fp16,
                                                     tag=f"aext{k}",
                                                     name=f"aext{k}")
                                nc.sync.dma_start(
                                    at[:], aextT[k * 128:(k + 1) * 128, :]
                                )
                                aext_tiles[k] = at
                    if h == 0 and tg == 0:
                        nc.sync.dma_start(h_tile[:], hmat[:, :])
                        nc.sync.dma_start(bias_tile[:], bias_bc[:, :])

                    # tt0's main matmuls are interleaved per-chunk with the
                    # u matmuls (pass 0) so the PE starts after chunk 0 lands.
                    ps0 = [
                        psum_pool.tile([128, 512], fp32, tag="mm",
                                       name=f"mm{h}_{tg}_0_{j}")
                        for j in range(4)
                    ]
                    if h == 0:
                        # up shares the mm pool slots (it is released by the
                        # gt copy before the 8th concurrent mm bank is needed)
                        up = psum_pool.tile([9, TG * 128], fp32, tag="mm",
                                            name=f"up{tg}")
                    for k in range(KC):
                        if h == 0:
                            nc.tensor.matmul(
                                up[:], aext_tiles[k][:], xs_tiles[(tg, k)][:],
                                start=(k == 0), stop=(k == KC - 1),
                            )
                        lhsT = xs_tiles[(tg, k)][:, 0:128]
                        for j in range(4):
                            nc.tensor.matmul(
                                ps0[j][:], lhsT,
                                nib_tiles[k][:, j * 512:(j + 1) * 512],
                                start=(k == 0), stop=False,
                            )
                    if h == 0:
                        gt = g_pool.tile([9, TG * 128], fp16, tag=f"g{tg}",
                                         bufs=1, name=f"g{tg}")
                        nc.vector.tensor_copy(gt[:, :], up[:])
                        gts[tg] = gt
                    gt = gts[tg]

                    for tt in range(TG):
                        if tt == 0:
                            ps = ps0
                        else:
                            ps = [
                                psum_pool.tile([128, 512], fp32, tag="mm",
                                               name=f"mm{h}_{tg}_{tt}_{j}")
                                for j in range(4)
                            ]
                            for k in range(KC):
                                lhsT = xs_tiles[(tg, k)][:,
                                                         tt * 128:(tt + 1) * 128]
                                for j in range(4):
                                    nc.tensor.matmul(
                                        ps[j][:], lhsT,
                                        nib_tiles[k][:, j * 512:(j + 1) * 512],
                                        start=(k == 0), stop=False,
                                    )
                        gs = gt[:, tt * 128:(tt + 1) * 128]
                        for j in range(4):
                            nc.tensor.matmul(
                                ps[j][:], gs,
                                h_tile[:, o0 + j * 512:o0 + (j + 1) * 512],
                                start=False, stop=True,
                            )
                        ot = out_pool.tile([128, OH], fp32, tag="out")
                        for j in range(4):
                            nc.vector.tensor_add(
                                ot[:, j * 512:(j + 1) * 512], ps[j][:],
                                bias_tile[:, o0 + j * 512:o0 + (j + 1) * 512],
                            )
                        trow = t0 + tt * 128
                        nc.sync.dma_start(y[trow:trow + 128, o0:o0 + OH], ot[:])
    nc.compile()
    return nc


def _prep_inputs(x, weight_quant, scale, zero, lora_A, lora_B, bias):
    """Host-side layout prep + sharding. Returns in_maps for 8 cores."""
    import ml_dtypes

    xs = (x.reshape(T, I).astype(np.float32) * scale[None, :]).astype(np.float16)
    xsT = np.ascontiguousarray(xs.T)  # [I, T]

    wq = weight_quant.astype(np.uint8)            # low byte only is populated
    nib = np.empty((O, I), np.uint8)
    nib[:, 0::2] = wq & 15
    nib[:, 1::2] = wq >> 4
    nibT = np.ascontiguousarray(nib.T.astype(ml_dtypes.float8_e4m3fn))  # [I, O]

    aextT = np.empty((I, 9), np.float16)
    aextT[:, 0:8] = (lora_A.astype(np.float32) / scale[None, :]).T
    aextT[:, 8] = zero
    aextT = np.ascontiguousarray(aextT)

    hmat = np.empty((9, O), np.float16)
    hmat[0:8, :] = 2.0 * lora_B.T
    hmat[8, :] = -1.0
    hmat = np.ascontiguousarray(hmat)
    bias_bc = np.ascontiguousarray(
        np.broadcast_to(bias.astype(np.float32), (128, O))
    )

    in_maps = []
    for c in range(NCORES):
        in_maps.append({
            "xsT": np.ascontiguousarray(xsT[:, c * TC:(c + 1) * TC]),
            "nibT": nibT,
            "aextT": aextT,
            "hmat": hmat,
            "bias_bc": bias_bc,
        })
    return in_maps


def run_on_cores(in_maps, trace=False):
    from concourse.bass_utils import run_bass_kernel_spmd

    if "nc" not in _CACHE:
        _CACHE["nc"] = _build_program()
    return run_bass_kernel_spmd(
        _CACHE["nc"], in_maps, list(range(NCORES)), trace=trace
    )


def kernel(x, weight_quant, scale, zero, lora_A, lora_B, bias):
    x = np.asarray(x)
    weight_quant = np.asarray(weight_quant)
    scale = np.asarray(scale, np.float32)
    zero = np.asarray(zero, np.float32)
    lora_A = np.asarray(lora_A, np.float32)
    lora_B = np.asarray(lora_B, np.float32)
    bias = np.asarray(bias, np.float32)

    in_maps = _prep_inputs(x, weight_quant, scale, zero, lora_A, lora_B, bias)
    res = run_on_cores(in_maps).results

    out = np.concatenate([res[c]["y"] for c in range(NCORES)], axis=0)
    return np.ascontiguousarray(out).reshape(B, S, O)



# revision 2
# speedup vs baseline: 1.7847x; 1.7847x over previous
"""Quantized (4-bit) LoRA linear for Trainium2, SPMD over 8 NeuronCores.

Math:  y[t,o] = sum_i x[t,i]*W[o,i] + bias[o] + 2.0 * sum_r (x@A^T)[t,r]*B[o,r]
where  W[o,i] = (nib[o,i] - zero[i]) * scale[i],  nib = unpacked 4-bit ints.

Strategy (fp8 DoubleRow): with xsF[t,i] = x[t,i]*scale[i]*F (F=128, lifts
values out of the fp8 subnormal range), split xsF = hi + lo into two
fp8e4m3 operands (error-feedback split, quantization error ~1e-3).  Then

  F*y[t,o] = sum_i (hi+lo)[t,i]*nib[o,i]        (fp8 x fp8 DoubleRow matmuls,
                                                 nib in {0..15} exact in fp8)
           + G[t,:] @ H[:,o]                    (K=17 fp16 matmul)

  G cols 0-7 = F*(x@A^T)[t,r]   (from fp8 aext = A/scale, DoubleRow)
        8,9  = sum_i xsF*z_h, sum_i xsF*z_l  (zero split hi/lo in fp8)
        10-15 = unused (pad: DoubleRow stationary needs M=16)
        16   = 1.0 (const row -> bias)
  H rows 0-7 = 2*B^T, 8,9 = -1, 10-15 = 0, 16 = F*bias

DoubleRow contracts 2 k-tiles (256 rows of I) per instruction at 0.5
cycles/row = 4x the fp16 matmul rate; two passes (hi+lo) -> net 2x.

Sharding: 8-way token split (1024 tokens/core); each core computes all 4096
outs in two o-halves (nib half streamed through SBUF, xs resident).
Emission is j-major (k-pair-major) within token-tile groups so the PE
pipelines directly behind the streaming DMAs, and each group's LoRA/eviction
work is deferred behind the next group's matmuls to avoid PE bubbles.
Output is written fp16 (cast to fp32 on host).
"""

import numpy as np

B, S, I, O = 4, 2048, 4096, 4096
T = B * S            # 8192 tokens
NCORES = 8
TC = T // NCORES     # 1024 tokens per core
KP = I // 256        # 16 k-pairs (DoubleRow contracts 256 rows each)
OH = O // 2          # o-half width
NTT = TC // 128      # 8 token tiles per core
NS = OH // 512       # 4 psum strips per o-half
F = 128.0            # fp8 pre-scale (power of two, exact)

_CACHE = {}


def _build_program():
    import concourse.bacc as bacc
    import concourse.mybir as mybir
    import concourse.tile as tile

    fp16 = mybir.dt.float16
    fp32 = mybir.dt.float32
    fp8 = mybir.dt.float8e4
    DR = mybir.MatmulPerfMode.DoubleRow

    nc = bacc.Bacc("TRN2", target_bir_lowering=False, debug=False)
    xs8 = nc.dram_tensor("xs8", [KP, 128, 2, 2, TC], fp8, kind="ExternalInput")
    nib8 = nc.dram_tensor("nib8", [KP, 128, 2, O], fp8, kind="ExternalInput")
    aext8 = nc.dram_tensor("aext8", [KP, 128, 2, 16], fp8, kind="ExternalInput")
    hmat = nc.dram_tensor("hmat", [17, O], fp16, kind="ExternalInput")
    y = nc.dram_tensor("y", [TC, O], fp16, kind="ExternalOutput")

    with tile.TileContext(nc) as tc:
        with (
            tc.tile_pool(name="xs", bufs=1) as xs_pool,
            tc.tile_pool(name="nib", bufs=22) as nib_pool,
            tc.tile_pool(name="consts", bufs=1) as const_pool,
            tc.tile_pool(name="g", bufs=1) as g_pool,
            tc.tile_pool(name="out", bufs=3) as out_pool,
            tc.tile_pool(name="psum", bufs=8, space="PSUM") as psum_pool,
        ):
            h_tile = const_pool.tile([17, O], fp16, tag="h")
            gt = g_pool.tile([17, TC], fp16, tag="g", name="gt")
            xs_tiles = [None] * KP
            aext_tiles = [None] * KP
            nib_tiles = {}  # (j, half) -> tile

            def load_nib(j, h):
                nt = nib_pool.tile([128, 2, OH], fp8, tag="nib",
                                   name=f"nib{h}_{j}")
                nc.sync.dma_start(nt[:], nib8[j][:, :, h * OH:(h + 1) * OH])
                nib_tiles[(j, h)] = nt

            def main_insts(j, h, tt, ps, first):
                """The 4 DoubleRow insts of k-pair j for chain (h,tt,s)."""
                for s in range(NS):
                    for hl in range(2):
                        for c in range(2):
                            nc.tensor.matmul(
                                ps[s][:, c * 256:(c + 1) * 256],
                                xs_tiles[j][:, :, hl,
                                            tt * 128:(tt + 1) * 128],
                                nib_tiles[(j, h)][:, :,
                                                  s * 512 + c * 256:
                                                  s * 512 + (c + 1) * 256],
                                start=(first and hl == 0 and c == 0),
                                stop=False, perf_mode=DR,
                            )

            def finish_tt(h, tt, ps):
                """LoRA/zero/bias matmul + eviction + store for chain group."""
                ot = out_pool.tile([128, OH], fp16, tag="out",
                                   name=f"ot{h}_{tt}")
                for s in range(NS):
                    nc.tensor.matmul(
                        ps[s][:, :], gt[:, tt * 128:(tt + 1) * 128],
                        h_tile[:, h * OH + s * 512:h * OH + (s + 1) * 512],
                        start=False, stop=True,
                    )
                for s in range(NS):
                    nc.vector.tensor_scalar_mul(
                        ot[:, s * 512:(s + 1) * 512], ps[s][:, :], 1.0 / F)
                nc.scalar.dma_start(
                    y[tt * 128:(tt + 1) * 128, h * OH:(h + 1) * OH], ot[:])

            def new_ps(h, tt):
                return [psum_pool.tile([128, 512], fp32, tag="mm",
                                       name=f"ps{h}_{tt}_{s}")
                        for s in range(NS)]

            # ---- group 0 (half 0, tt 0) + u chains, j-major behind DMAs ----
            up = [psum_pool.tile([16, 512], fp32, tag="mm", name=f"up{uc}")
                  for uc in range(2)]
            ps_prev = new_ps(0, 0)
            for j in range(KP):
                xt = xs_pool.tile([128, 2, 2, TC], fp8, tag=f"xs{j}",
                                  name=f"xs{j}")
                nc.sync.dma_start(xt[:], xs8[j])
                xs_tiles[j] = xt
                at = const_pool.tile([128, 2, 16], fp8, tag=f"aext{j}",
                                     name=f"aext{j}")
                nc.sync.dma_start(at[:], aext8[j])
                aext_tiles[j] = at
                if j == 0:
                    nc.sync.dma_start(h_tile[:], hmat[:, :])
                    nc.vector.memset(gt[:, :], 1.0)
                load_nib(j, 0)
                for uc in range(2):
                    for hl in range(2):
                        for c in range(2):
                            nc.tensor.matmul(
                                up[uc][:, c * 256:(c + 1) * 256],
                                aext_tiles[j][:],
                                xs_tiles[j][:, :, hl,
                                            uc * 512 + c * 256:
                                            uc * 512 + (c + 1) * 256],
                                start=(j == 0 and hl == 0 and c == 0),
                                stop=(j == KP - 1 and hl == 1 and c == 1),
                                perf_mode=DR,
                            )
                main_insts(j, 0, 0, ps_prev, first=(j == 0))
            for uc in range(2):
                nc.vector.tensor_copy(gt[0:16, uc * 512:(uc + 1) * 512],
                                      up[uc][:, :])
            # early prefetch of half-1 nib into the 6 spare pool slots
            for j in range(6):
                load_nib(j, 1)

            # ---- remaining groups, h/evict work staggered one group back ---
            tt_prev = 0
            for h in range(2):
                for tt in range(NTT):
                    if h == 0 and tt == 0:
                        continue
                    ps = new_ps(h, tt)
                    for j in range(KP):
                        if h == 1 and tt == 0 and j >= 6:
                            load_nib(j, 1)
                        main_insts(j, h, tt, ps, first=(j == 0))
                    finish_tt(h if tt > 0 else 0, tt_prev, ps_prev)
                    ps_prev, tt_prev = ps, tt
            finish_tt(1, tt_prev, ps_prev)
    nc.compile()
    return nc


def _prep_inputs(x, weight_quant, scale, zero, lora_A, lora_B, bias):
    """Host-side layout prep + sharding. Returns in_maps for 8 cores."""
    import ml_dtypes

    e4 = ml_dtypes.float8_e4m3fn

    # xsF = x*scale*F, split hi/lo fp8, laid out [KP, 128, pair, hi/lo, T]
    xsF = (x.reshape(T, I).astype(np.float32) * (scale[None, :] * F))
    hi = xsF.astype(e4)
    lo = (xsF - hi.astype(np.float32)).astype(e4)

    def kshuf(arr):  # [I, ...] -> [KP, 128, 2, ...]
        return np.ascontiguousarray(
            arr.reshape(KP, 2, 128, *arr.shape[1:]).swapaxes(1, 2))

    xs8 = np.empty((KP, 128, 2, 2, T), e4)
    xs8[:, :, :, 0, :] = kshuf(np.ascontiguousarray(hi.T))
    xs8[:, :, :, 1, :] = kshuf(np.ascontiguousarray(lo.T))

    wq = weight_quant.astype(np.uint8)           # low byte only is populated
    nib = np.empty((O, I), np.uint8)
    nib[:, 0::2] = wq & 15
    nib[:, 1::2] = wq >> 4
    nib8 = kshuf(np.ascontiguousarray(nib.T).astype(e4))   # [KP,128,2,O]

    aext = np.zeros((I, 16), np.float32)
    aext[:, 0:8] = (lora_A.astype(np.float32) / scale[None, :]).T
    z_h = zero.astype(e4)
    z_l = (zero - z_h.astype(np.float32)).astype(e4)
    aext[:, 8] = z_h.astype(np.float32)
    aext[:, 9] = z_l.astype(np.float32)
    aext8 = kshuf(aext.astype(e4))                         # [KP,128,2,16]

    hmat = np.zeros((17, O), np.float16)
    hmat[0:8] = 2.0 * lora_B.T
    hmat[8] = -1.0
    hmat[9] = -1.0
    hmat[16] = F * bias
    hmat = np.ascontiguousarray(hmat)

    in_maps = []
    for c in range(NCORES):
        in_maps.append({
            "xs8": np.ascontiguousarray(xs8[..., c * TC:(c + 1) * TC]),
            "nib8": nib8,
            "aext8": aext8,
            "hmat": hmat,
        })
    return in_maps


def run_on_cores(in_maps, trace=False):
    from concourse.bass_utils import run_bass_kernel_spmd

    if "nc" not in _CACHE:
        _CACHE["nc"] = _build_program()
    return run_bass_kernel_spmd(
        _CACHE["nc"], in_maps, list(range(NCORES)), trace=trace
    )


def kernel(x, weight_quant, scale, zero, lora_A, lora_B, bias):
    x = np.asarray(x)
    weight_quant = np.asarray(weight_quant)
    scale = np.asarray(scale, np.float32)
    zero = np.asarray(zero, np.float32)
    lora_A = np.asarray(lora_A, np.float32)
    lora_B = np.asarray(lora_B, np.float32)
    bias = np.asarray(bias, np.float32)

    in_maps = _prep_inputs(x, weight_quant, scale, zero, lora_A, lora_B, bias)
    res = run_on_cores(in_maps).results

    out = np.concatenate(
        [res[c]["y"].astype(np.float32) for c in range(NCORES)], axis=0)
    return np.ascontiguousarray(out).reshape(B, S, O)


# revision 3
# speedup vs baseline: 1.8082x; 1.0132x over previous
"""Quantized (4-bit) LoRA linear for Trainium2, SPMD over 8 NeuronCores.

Math:  y[t,o] = sum_i x[t,i]*W[o,i] + bias[o] + 2.0 * sum_r (x@A^T)[t,r]*B[o,r]
where  W[o,i] = (nib[o,i] - zero[i]) * scale[i],  nib = unpacked 4-bit ints.

Strategy (fp8 DoubleRow): with xsF[t,i] = x[t,i]*scale[i]*F (F=128, lifts
values out of the fp8 subnormal range), split xsF = hi + lo into two
fp8e4m3 operands (error-feedback split, quantization error ~1e-3).  Then

  F*y[t,o] = sum_i (hi+lo)[t,i]*nib[o,i]        (fp8 x fp8 DoubleRow matmuls,
                                                 nib in {0..15} exact in fp8)
           + G[t,:] @ H[:,o]                    (K=17 fp16 matmul)

  G cols 0-7 = F*(x@A^T)[t,r]   (from fp8 aext = A/scale, DoubleRow)
        8,9  = sum_i xsF*z_h, sum_i xsF*z_l  (zero split hi/lo in fp8)
        10-15 = unused (pad: DoubleRow stationary needs M=16)
        16   = 1.0 (const row -> bias)
  H rows 0-7 = 2*B^T, 8,9 = -1, 10-15 = 0, 16 = F*bias

DoubleRow contracts 2 k-tiles (256 rows of I) per instruction at 0.5
cycles/row = 4x the fp16 matmul rate; two passes (hi+lo) -> net 2x.

Sharding: 8-way token split (1024 tokens/core); each core computes all 4096
outs in two o-halves (nib half streamed through SBUF, xs resident).
Emission is j-major (k-pair-major) within token-tile groups so the PE
pipelines directly behind the streaming DMAs, and each group's LoRA/eviction
work is deferred behind the next group's matmuls to avoid PE bubbles.
Output is written fp16 (cast to fp32 on host).
"""

import numpy as np

B, S, I, O = 4, 2048, 4096, 4096
T = B * S            # 8192 tokens
NCORES = 8
TC = T // NCORES     # 1024 tokens per core
KP = I // 256        # 16 k-pairs (DoubleRow contracts 256 rows each)
OH = O // 2          # o-half width
NTT = TC // 128      # 8 token tiles per core
NS = OH // 512       # 4 psum strips per o-half
F = 128.0            # fp8 pre-scale (power of two, exact)

_CACHE = {}


def _build_program():
    import concourse.bacc as bacc
    import concourse.mybir as mybir
    import concourse.tile as tile

    fp16 = mybir.dt.float16
    fp32 = mybir.dt.float32
    fp8 = mybir.dt.float8e4
    DR = mybir.MatmulPerfMode.DoubleRow

    nc = bacc.Bacc("TRN2", target_bir_lowering=False, debug=False)
    xs8 = nc.dram_tensor("xs8", [KP, 128, 2, 2, TC], fp8, kind="ExternalInput")
    nib8 = nc.dram_tensor("nib8", [KP, 128, 2, O], fp8, kind="ExternalInput")
    aext8 = nc.dram_tensor("aext8", [KP, 128, 2, 16], fp8, kind="ExternalInput")
    hmat = nc.dram_tensor("hmat", [17, O], fp16, kind="ExternalInput")
    y = nc.dram_tensor("y", [TC, O], fp16, kind="ExternalOutput")

    with tile.TileContext(nc) as tc:
        with (
            tc.tile_pool(name="xs", bufs=1) as xs_pool,
            tc.tile_pool(name="nib", bufs=22) as nib_pool,
            tc.tile_pool(name="consts", bufs=1) as const_pool,
            tc.tile_pool(name="g", bufs=1) as g_pool,
            tc.tile_pool(name="out", bufs=3) as out_pool,
            tc.tile_pool(name="psum", bufs=8, space="PSUM") as psum_pool,
        ):
            h_tile = const_pool.tile([17, O], fp16, tag="h")
            gt = g_pool.tile([17, TC], fp16, tag="g", name="gt")
            xs_tiles = [None] * KP
            aext_tiles = [None] * KP
            nib_tiles = {}  # (j, half) -> tile

            def load_nib(j, h):
                nt = nib_pool.tile([128, 2, OH], fp8, tag="nib",
                                   name=f"nib{h}_{j}")
                nc.sync.dma_start(nt[:], nib8[j][:, :, h * OH:(h + 1) * OH])
                nib_tiles[(j, h)] = nt

            def main_insts(j, h, tt, ps, first):
                """The 4 DoubleRow insts of k-pair j for chain (h,tt,s)."""
                for s in range(NS):
                    for hl in range(2):
                        for c in range(2):
                            nc.tensor.matmul(
                                ps[s][:, c * 256:(c + 1) * 256],
                                xs_tiles[j][:, :, hl,
                                            tt * 128:(tt + 1) * 128],
                                nib_tiles[(j, h)][:, :,
                                                  s * 512 + c * 256:
                                                  s * 512 + (c + 1) * 256],
                                start=(first and hl == 0 and c == 0),
                                stop=False, perf_mode=DR,
                            )

            def finish_tt(h, tt, ps):
                """LoRA/zero/bias matmul + eviction + store for chain group."""
                ot = out_pool.tile([128, OH], fp16, tag="out",
                                   name=f"ot{h}_{tt}")
                for s in range(NS):
                    nc.tensor.matmul(
                        ps[s][:, :], gt[:, tt * 128:(tt + 1) * 128],
                        h_tile[:, h * OH + s * 512:h * OH + (s + 1) * 512],
                        start=False, stop=True,
                    )
                for s in range(NS):
                    nc.vector.tensor_scalar_mul(
                        ot[:, s * 512:(s + 1) * 512], ps[s][:, :], 1.0 / F)
                nc.scalar.dma_start(
                    y[tt * 128:(tt + 1) * 128, h * OH:(h + 1) * OH], ot[:])

            def new_ps(h, tt):
                return [psum_pool.tile([128, 512], fp32, tag="mm",
                                       name=f"ps{h}_{tt}_{s}")
                        for s in range(NS)]

            # ---- group 0 (half 0, tt 0 + first half of tt 1) + u chains ----
            # j-major emission right behind the streaming DMAs; 8 PSUM banks
            # all open (2 u + 4 tt0 + 2 tt1) to maximize PE fill while the
            # 16 MB of phase-A DMA (xs + nib half 0) streams in.
            up = [psum_pool.tile([16, 512], fp32, tag="mm", name=f"up{uc}")
                  for uc in range(2)]
            ps_prev = new_ps(0, 0)
            ps_tt1 = new_ps(0, 1)  # s0/s1 filled in group 0, s2/s3 in group 1
            for j in range(KP):
                at = const_pool.tile([128, 2, 16], fp8, tag=f"aext{j}",
                                     name=f"aext{j}")
                nc.sync.dma_start(at[:], aext8[j])
                aext_tiles[j] = at
                xt = xs_pool.tile([128, 2, 2, TC], fp8, tag=f"xs{j}",
                                  name=f"xs{j}")
                nc.sync.dma_start(xt[:], xs8[j])
                xs_tiles[j] = xt
                if j == 0:
                    nc.sync.dma_start(h_tile[:], hmat[:, :])
                    nc.vector.memset(gt[:, :], 1.0)
                load_nib(j, 0)
                for uc in range(2):
                    for hl in range(2):
                        for c in range(2):
                            nc.tensor.matmul(
                                up[uc][:, c * 256:(c + 1) * 256],
                                aext_tiles[j][:],
                                xs_tiles[j][:, :, hl,
                                            uc * 512 + c * 256:
                                            uc * 512 + (c + 1) * 256],
                                start=(j == 0 and hl == 0 and c == 0),
                                stop=(j == KP - 1 and hl == 1 and c == 1),
                                perf_mode=DR,
                            )
                main_insts(j, 0, 0, ps_prev, first=(j == 0))
                for s in range(2):
                    for hl in range(2):
                        for c in range(2):
                            nc.tensor.matmul(
                                ps_tt1[s][:, c * 256:(c + 1) * 256],
                                xs_tiles[j][:, :, hl, 128:256],
                                nib_tiles[(j, 0)][:, :,
                                                  s * 512 + c * 256:
                                                  s * 512 + (c + 1) * 256],
                                start=(j == 0 and hl == 0 and c == 0),
                                stop=False, perf_mode=DR,
                            )
            for uc in range(2):
                nc.vector.tensor_copy(gt[0:16, uc * 512:(uc + 1) * 512],
                                      up[uc][:, :])
            # early prefetch of half-1 nib into the 6 spare pool slots
            for j in range(6):
                load_nib(j, 1)
            # group 1: finish tt1 (strips 2,3 only)
            for j in range(KP):
                for s in (2, 3):
                    for hl in range(2):
                        for c in range(2):
                            nc.tensor.matmul(
                                ps_tt1[s][:, c * 256:(c + 1) * 256],
                                xs_tiles[j][:, :, hl, 128:256],
                                nib_tiles[(j, 0)][:, :,
                                                  s * 512 + c * 256:
                                                  s * 512 + (c + 1) * 256],
                                start=(j == 0 and hl == 0 and c == 0),
                                stop=False, perf_mode=DR,
                            )
            finish_tt(0, 0, ps_prev)
            ps_prev, tt_prev = ps_tt1, 1

            # ---- remaining groups, h/evict work staggered one group back ---
            for h in range(2):
                for tt in range(NTT):
                    if h == 0 and tt <= 1:
                        continue
                    if h == 1 and tt == NTT - 1:
                        break  # last group handled strip-major below
                    ps = new_ps(h, tt)
                    for j in range(KP):
                        if h == 1 and tt == 0 and j >= 6:
                            load_nib(j, 1)
                        main_insts(j, h, tt, ps, first=(j == 0))
                    finish_tt(h if tt > 0 else 0, tt_prev, ps_prev)
                    ps_prev, tt_prev = ps, tt

            # ---- last group (h=1, tt=7): strip-major so the tail drains
            # strip-by-strip instead of all-at-once after the final matmul.
            h, tt = 1, NTT - 1
            ps = new_ps(h, tt)
            ot_last = out_pool.tile([128, OH], fp16, tag="out", name="ot_last")
            for s in range(NS):
                for j in range(KP):
                    for hl in range(2):
                        for c in range(2):
                            nc.tensor.matmul(
                                ps[s][:, c * 256:(c + 1) * 256],
                                xs_tiles[j][:, :, hl,
                                            tt * 128:(tt + 1) * 128],
                                nib_tiles[(j, h)][:, :,
                                                  s * 512 + c * 256:
                                                  s * 512 + (c + 1) * 256],
                                start=(j == 0 and hl == 0 and c == 0),
                                stop=False, perf_mode=DR,
                            )
                if s == 0:
                    finish_tt(1, tt_prev, ps_prev)
                nc.tensor.matmul(
                    ps[s][:, :], gt[:, tt * 128:(tt + 1) * 128],
                    h_tile[:, h * OH + s * 512:h * OH + (s + 1) * 512],
                    start=False, stop=True,
                )
                nc.vector.tensor_scalar_mul(
                    ot_last[:, s * 512:(s + 1) * 512], ps[s][:, :], 1.0 / F)
                nc.scalar.dma_start(
                    y[tt * 128:(tt + 1) * 128,
                      h * OH + s * 512:h * OH + (s + 1) * 512],
                    ot_last[:, s * 512:(s + 1) * 512])
    nc.compile()
    return nc


def _prep_inputs(x, weight_quant, scale, zero, lora_A, lora_B, bias):
    """Host-side layout prep + sharding. Returns in_maps for 8 cores."""
    import ml_dtypes

    e4 = ml_dtypes.float8_e4m3fn

    # xsF = x*scale*F, split hi/lo fp8, laid out [KP, 128, pair, hi/lo, T]
    xsF = (x.reshape(T, I).astype(np.float32) * (scale[None, :] * F))
    hi = xsF.astype(e4)
    lo = (xsF - hi.astype(np.float32)).astype(e4)

    def kshuf(arr):  # [I, ...] -> [KP, 128, 2, ...]
        return np.ascontiguousarray(
            arr.reshape(KP, 2, 128, *arr.shape[1:]).swapaxes(1, 2))

    xs8 = np.empty((KP, 128, 2, 2, T), e4)
    xs8[:, :, :, 0, :] = kshuf(np.ascontiguousarray(hi.T))
    xs8[:, :, :, 1, :] = kshuf(np.ascontiguousarray(lo.T))

    wq = weight_quant.astype(np.uint8)           # low byte only is populated
    nib = np.empty((O, I), np.uint8)
    nib[:, 0::2] = wq & 15
    nib[:, 1::2] = wq >> 4
    nib8 = kshuf(np.ascontiguousarray(nib.T).astype(e4))   # [KP,128,2,O]

    aext = np.zeros((I, 16), np.float32)
    aext[:, 0:8] = (lora_A.astype(np.float32) / scale[None, :]).T
    z_h = zero.astype(e4)
    z_l = (zero - z_h.astype(np.float32)).astype(e4)
    aext[:, 8] = z_h.astype(np.float32)
    aext[:, 9] = z_l.astype(np.float32)
    aext8 = kshuf(aext.astype(e4))                         # [KP,128,2,16]

    hmat = np.zeros((17, O), np.float16)
    hmat[0:8] = 2.0 * lora_B.T
    hmat[8] = -1.0
    hmat[9] = -1.0
    hmat[16] = F * bias
    hmat = np.ascontiguousarray(hmat)

    in_maps = []
    for c in range(NCORES):
        in_maps.append({
            "xs8": np.ascontiguousarray(xs8[..., c * TC:(c + 1) * TC]),
            "nib8": nib8,
            "aext8": aext8,
            "hmat": hmat,
        })
    return in_maps


def run_on_cores(in_maps, trace=False):
    from concourse.bass_utils import run_bass_kernel_spmd

    if "nc" not in _CACHE:
        _CACHE["nc"] = _build_program()
    return run_bass_kernel_spmd(
        _CACHE["nc"], in_maps, list(range(NCORES)), trace=trace
    )


def kernel(x, weight_quant, scale, zero, lora_A, lora_B, bias):
    x = np.asarray(x)
    weight_quant = np.asarray(weight_quant)
    scale = np.asarray(scale, np.float32)
    zero = np.asarray(zero, np.float32)
    lora_A = np.asarray(lora_A, np.float32)
    lora_B = np.asarray(lora_B, np.float32)
    bias = np.asarray(bias, np.float32)

    in_maps = _prep_inputs(x, weight_quant, scale, zero, lora_A, lora_B, bias)
    res = run_on_cores(in_maps).results

    out = np.concatenate(
        [res[c]["y"].astype(np.float32) for c in range(NCORES)], axis=0)
    return np.ascontiguousarray(out).reshape(B, S, O)


# revision 8
# speedup vs baseline: 1.8413x; 1.0183x over previous
"""Quantized (4-bit) LoRA linear for Trainium2, SPMD over 8 NeuronCores.

Math:  y[t,o] = sum_i x[t,i]*W[o,i] + bias[o] + 2.0 * sum_r (x@A^T)[t,r]*B[o,r]
where  W[o,i] = (nib[o,i] - zero[i]) * scale[i],  nib = unpacked 4-bit ints.

Strategy (fp8 DoubleRow): with xsF[t,i] = x[t,i]*scale[i]*F (F=128, lifts
values out of the fp8 subnormal range), split xsF = hi + lo into two
fp8e4m3 operands (error-feedback split, quantization error ~1e-3).  Then

  F*y[t,o] = sum_i (hi+lo)[t,i]*nib[o,i]        (fp8 x fp8 DoubleRow matmuls,
                                                 nib in {0..15} exact in fp8)
           + G[t,:] @ H[:,o]                    (K=17 fp16 matmul)

  G cols 0-7 = F*(x@A^T)[t,r]   (from fp8 aext = A/scale, DoubleRow)
        8,9  = sum_i xsF*z_h, sum_i xsF*z_l  (zero split hi/lo in fp8)
        10-15 = unused (pad: DoubleRow stationary needs M=16)
        16   = 1.0 (const row -> bias)
  H rows 0-7 = 2*B^T, 8,9 = -1, 10-15 = 0, 16 = F*bias

DoubleRow contracts 2 k-tiles (256 rows of I) per instruction at 0.5
cycles/row = 4x the fp16 matmul rate; two passes (hi+lo) -> net 2x.

Sharding: 8-way token split (1024 tokens/core); each core computes all 4096
outs in two o-halves (nib half streamed through SBUF, xs resident).
Emission is j-major (k-pair-major) within token-tile groups so the PE
pipelines directly behind the streaming DMAs, and each group's LoRA/eviction
work is deferred behind the next group's matmuls to avoid PE bubbles.
Output is written fp16 (cast to fp32 on host).
"""

import numpy as np

B, S, I, O = 4, 2048, 4096, 4096
T = B * S            # 8192 tokens
NCORES = 8
TC = T // NCORES     # 1024 tokens per core
KP = I // 256        # 16 k-pairs (DoubleRow contracts 256 rows each)
OH = O // 2          # o-half width
NTT = TC // 128      # 8 token tiles per core
NS = OH // 512       # 4 psum strips per o-half
F = 128.0            # fp8 pre-scale (power of two, exact)

_CACHE = {}


def _build_program():
    import concourse.bacc as bacc
    import concourse.mybir as mybir
    import concourse.tile as tile

    fp16 = mybir.dt.float16
    fp32 = mybir.dt.float32
    fp8 = mybir.dt.float8e4
    DR = mybir.MatmulPerfMode.DoubleRow

    nc = bacc.Bacc("TRN2", target_bir_lowering=False, debug=False)
    xs8 = nc.dram_tensor("xs8", [KP, 128, 2, 2, TC], fp8, kind="ExternalInput")
    nib8 = nc.dram_tensor("nib8", [KP, 128, 2, O], fp8, kind="ExternalInput")
    aext8 = nc.dram_tensor("aext8", [KP, 128, 2, 16], fp8, kind="ExternalInput")
    h8 = nc.dram_tensor("h8", [10, 2, O], fp8, kind="ExternalInput")
    y = nc.dram_tensor("y", [TC, O], fp16, kind="ExternalOutput")

    with tile.TileContext(nc) as tc:
        with (
            tc.tile_pool(name="xs", bufs=1) as xs_pool,
            tc.tile_pool(name="nib", bufs=22) as nib_pool,
            tc.tile_pool(name="consts", bufs=1) as const_pool,
            tc.tile_pool(name="g", bufs=1) as g_pool,
            tc.tile_pool(name="out", bufs=3) as out_pool,
            tc.tile_pool(name="psum", bufs=8, space="PSUM") as psum_pool,
        ):
            h_tile = const_pool.tile([10, 2, O], fp8, tag="h")
            uf16 = g_pool.tile([16, TC], fp16, tag="uf", name="uf16")
            gt8 = g_pool.tile([10, 2, TC], fp8, tag="g", name="gt8")
            ct_tiles = [const_pool.tile([128, 16], fp16, tag=f"ct{tt}",
                                        name=f"ct{tt}") for tt in range(NTT)]
            ct32_tiles = [const_pool.tile([128, 1], fp32, tag=f"ct32_{tt}",
                                          name=f"ct32_{tt}")
                          for tt in range(NTT)]
            xs_tiles = [None] * KP
            aext_tiles = [None] * KP
            nib_tiles = {}  # (j, half) -> tile
            nc.vector.memset(gt8[:, :, :], 0.0)
            nc.vector.memset(gt8[:, 0, :], 1.0)  # const row (bias)

            def load_nib(j, h):
                nt = nib_pool.tile([128, 2, OH], fp8, tag="nib",
                                   name=f"nib{h}_{j}")
                nc.sync.dma_start(nt[:], nib8[j][:, :, h * OH:(h + 1) * OH])
                nib_tiles[(j, h)] = nt

            def main_insts(j, h, tt, ps, first):
                """The 4 DoubleRow insts of k-pair j for chain (h,tt,s)."""
                for s in range(NS):
                    for hl in range(2):
                        for c in range(2):
                            nc.tensor.matmul(
                                ps[s][:, c * 256:(c + 1) * 256],
                                xs_tiles[j][:, :, hl,
                                            tt * 128:(tt + 1) * 128],
                                nib_tiles[(j, h)][:, :,
                                                  s * 512 + c * 256:
                                                  s * 512 + (c + 1) * 256],
                                start=(first and hl == 0 and c == 0),
                                stop=False, perf_mode=DR,
                            )

            def finish_tt(h, tt, ps):
                """LoRA/zero/bias matmul + eviction + store for chain group."""
                ot = out_pool.tile([128, OH], fp16, tag="out",
                                   name=f"ot{h}_{tt}")
                for s in range(NS):
                    for c in range(2):
                        off = h * OH + s * 512 + c * 256
                        nc.tensor.matmul(
                            ps[s][:, c * 256:(c + 1) * 256],
                            gt8[:, :, tt * 128:(tt + 1) * 128],
                            h_tile[:, :, off:off + 256],
                            start=False, stop=(c == 1), perf_mode=DR,
                        )
                for s in range(NS):
                    nc.vector.tensor_scalar(
                        ot[:, s * 512:(s + 1) * 512], ps[s][:, :],
                        ct32_tiles[tt][:, :], 1.0 / F,
                        op0=mybir.AluOpType.subtract,
                        op1=mybir.AluOpType.mult)
                nc.scalar.dma_start(
                    y[tt * 128:(tt + 1) * 128, h * OH:(h + 1) * OH], ot[:])

            def new_ps(h, tt):
                return [psum_pool.tile([128, 512], fp32, tag="mm",
                                       name=f"ps{h}_{tt}_{s}")
                        for s in range(NS)]

            # ---- group 0 (half 0, tt 0 + first half of tt 1) + u chains ----
            # j-major emission right behind the streaming DMAs; 8 PSUM banks
            # all open (2 u + 4 tt0 + 2 tt1) to maximize PE fill while the
            # 16 MB of phase-A DMA (xs + nib half 0) streams in.
            up = [psum_pool.tile([16, 512], fp32, tag="mm", name=f"up{uc}")
                  for uc in range(2)]
            ps_prev = new_ps(0, 0)
            ps_tt1 = new_ps(0, 1)  # s0/s1 filled in group 0, s2/s3 in group 1
            for j in range(KP):
                at = const_pool.tile([128, 2, 16], fp8, tag=f"aext{j}",
                                     name=f"aext{j}")
                nc.sync.dma_start(at[:], aext8[j])
                aext_tiles[j] = at
                xt = xs_pool.tile([128, 2, 2, TC], fp8, tag=f"xs{j}",
                                  name=f"xs{j}")
                nc.sync.dma_start(xt[:], xs8[j])
                xs_tiles[j] = xt
                if j == 0:
                    nc.sync.dma_start(h_tile[:], h8[:, :, :])
                load_nib(j, 0)
                for uc in range(2):
                    for hl in range(2):
                        for c in range(2):
                            nc.tensor.matmul(
                                up[uc][:, c * 256:(c + 1) * 256],
                                aext_tiles[j][:],
                                xs_tiles[j][:, :, hl,
                                            uc * 512 + c * 256:
                                            uc * 512 + (c + 1) * 256],
                                start=(j == 0 and hl == 0 and c == 0),
                                stop=(j == KP - 1 and hl == 1 and c == 1),
                                perf_mode=DR,
                            )
                main_insts(j, 0, 0, ps_prev, first=(j == 0))
                for s in range(2):
                    for hl in range(2):
                        for c in range(2):
                            nc.tensor.matmul(
                                ps_tt1[s][:, c * 256:(c + 1) * 256],
                                xs_tiles[j][:, :, hl, 128:256],
                                nib_tiles[(j, 0)][:, :,
                                                  s * 512 + c * 256:
                                                  s * 512 + (c + 1) * 256],
                                start=(j == 0 and hl == 0 and c == 0),
                                stop=False, perf_mode=DR,
                            )
            for uc in range(2):
                nc.vector.tensor_copy(uf16[:, uc * 512:(uc + 1) * 512],
                                      up[uc][:, :])
            nc.vector.tensor_scalar_mul(gt8[0:9, 0, :], uf16[0:9, :], 1.0 / 64)
            for tt in range(NTT):
                nc.scalar.dma_start_transpose(
                    ct_tiles[tt][:, :], uf16[:, tt * 128:(tt + 1) * 128])
            for tt in range(NTT):
                nc.gpsimd.tensor_copy(ct32_tiles[tt][:, :],
                                      ct_tiles[tt][:, 9:10])
            # early prefetch of half-1 nib into the 6 spare pool slots
            for j in range(6):
                load_nib(j, 1)
            # group 1: finish tt1 (strips 2,3 only)
            for j in range(KP):
                for s in (2, 3):
                    for hl in range(2):
                        for c in range(2):
                            nc.tensor.matmul(
                                ps_tt1[s][:, c * 256:(c + 1) * 256],
                                xs_tiles[j][:, :, hl, 128:256],
                                nib_tiles[(j, 0)][:, :,
                                                  s * 512 + c * 256:
                                                  s * 512 + (c + 1) * 256],
                                start=(j == 0 and hl == 0 and c == 0),
                                stop=False, perf_mode=DR,
                            )
            finish_tt(0, 0, ps_prev)
            ps_prev, tt_prev = ps_tt1, 1

            # ---- remaining groups, h/evict work staggered one group back ---
            for h in range(2):
                for tt in range(NTT):
                    if h == 0 and tt <= 1:
                        continue
                    if h == 1 and tt == NTT - 1:
                        break  # last group handled strip-major below
                    ps = new_ps(h, tt)
                    for j in range(KP):
                        if h == 1 and tt == 0 and j >= 6:
                            load_nib(j, 1)
                        main_insts(j, h, tt, ps, first=(j == 0))
                    finish_tt(h if tt > 0 else 0, tt_prev, ps_prev)
                    ps_prev, tt_prev = ps, tt

            # ---- last group (h=1, tt=7): strip-major so the tail drains
            # strip-by-strip instead of all-at-once after the final matmul.
            h, tt = 1, NTT - 1
            ps = new_ps(h, tt)
            ot_last = out_pool.tile([128, OH], fp16, tag="out", name="ot_last")
            for s in range(NS):
                for j in range(KP):
                    for hl in range(2):
                        for c in range(2):
                            nc.tensor.matmul(
                                ps[s][:, c * 256:(c + 1) * 256],
                                xs_tiles[j][:, :, hl,
                                            tt * 128:(tt + 1) * 128],
                                nib_tiles[(j, h)][:, :,
                                                  s * 512 + c * 256:
                                                  s * 512 + (c + 1) * 256],
                                start=(j == 0 and hl == 0 and c == 0),
                                stop=False, perf_mode=DR,
                            )
                if s == 0:
                    finish_tt(1, tt_prev, ps_prev)
                for c in range(2):
                    off = h * OH + s * 512 + c * 256
                    nc.tensor.matmul(
                        ps[s][:, c * 256:(c + 1) * 256],
                        gt8[:, :, tt * 128:(tt + 1) * 128],
                        h_tile[:, :, off:off + 256],
                        start=False, stop=(c == 1), perf_mode=DR,
                    )
                nc.vector.tensor_scalar(
                    ot_last[:, s * 512:(s + 1) * 512], ps[s][:, :],
                    ct32_tiles[tt][:, :], 1.0 / F,
                    op0=mybir.AluOpType.subtract, op1=mybir.AluOpType.mult)
                nc.scalar.dma_start(
                    y[tt * 128:(tt + 1) * 128,
                      h * OH + s * 512:h * OH + (s + 1) * 512],
                    ot_last[:, s * 512:(s + 1) * 512])
    nc.compile()
    return nc


def _prep_inputs(x, weight_quant, scale, zero, lora_A, lora_B, bias):
    """Host-side layout prep + sharding. Returns in_maps for 8 cores."""
    import ml_dtypes

    e4 = ml_dtypes.float8_e4m3   # device fp8e4: e4m3 WITH inf, max 240

    # xsF = x*scale*F, split hi/lo fp8, laid out [KP, 128, pair, hi/lo, T]
    xsF = (x.reshape(T, I).astype(np.float32) * (scale[None, :] * F))
    hi = xsF.astype(e4)
    lo = (xsF - hi.astype(np.float32)).astype(e4)

    def kshuf(arr):  # [I, ...] -> [KP, 128, 2, ...]
        return np.ascontiguousarray(
            arr.reshape(KP, 2, 128, *arr.shape[1:]).swapaxes(1, 2))

    xs8 = np.empty((KP, 128, 2, 2, T), e4)
    xs8[:, :, :, 0, :] = kshuf(np.ascontiguousarray(hi.T))
    xs8[:, :, :, 1, :] = kshuf(np.ascontiguousarray(lo.T))

    wq = weight_quant.astype(np.uint8)           # low byte only is populated
    nib = np.empty((O, I), np.uint8)
    nib[:, 0::2] = wq & 15
    nib[:, 1::2] = wq >> 4
    nib8 = kshuf(np.ascontiguousarray(nib.T).astype(e4))   # [KP,128,2,O]

    # u-matmul columns: 0 = z_lo residual, 1-8 = A/scale, 9 = z_hi
    aext = np.zeros((I, 16), np.float32)
    z_h = zero.astype(e4)
    z_l = (zero - z_h.astype(np.float32)).astype(e4)
    aext[:, 0] = z_l.astype(np.float32)
    aext[:, 1:9] = (lora_A.astype(np.float32) / scale[None, :]).T
    aext[:, 9] = z_h.astype(np.float32)
    aext8 = kshuf(aext.astype(e4))                         # [KP,128,2,16]

    # h8 pair-0 rows contract gt8 pair-0 rows [c_l/64, u0..7/64, 1/64]
    h8 = np.zeros((10, 2, O), np.float32)
    h8[0, 0] = -64.0
    h8[1:9, 0] = 128.0 * lora_B.T          # 64 * 2 * B^T
    h8[9, 0] = F * bias                    # const row carries the bias
    assert np.abs(h8).max() < 240.0
    h8 = np.ascontiguousarray(h8.astype(e4))

    in_maps = []
    for c in range(NCORES):
        in_maps.append({
            "xs8": np.ascontiguousarray(xs8[..., c * TC:(c + 1) * TC]),
            "nib8": nib8,
            "aext8": aext8,
            "h8": h8,
        })
    return in_maps


def run_on_cores(in_maps, trace=False):
    from concourse.bass_utils import run_bass_kernel_spmd

    if "nc" not in _CACHE:
        _CACHE["nc"] = _build_program()
    return run_bass_kernel_spmd(
        _CACHE["nc"], in_maps, list(range(NCORES)), trace=trace
    )


def kernel(x, weight_quant, scale, zero, lora_A, lora_B, bias):
    x = np.asarray(x)
    weight_quant = np.asarray(weight_quant)
    scale = np.asarray(scale, np.float32)
    zero = np.asarray(zero, np.float32)
    lora_A = np.asarray(lora_A, np.float32)
    lora_B = np.asarray(lora_B, np.float32)
    bias = np.asarray(bias, np.float32)

    in_maps = _prep_inputs(x, weight_quant, scale, zero, lora_A, lora_B, bias)
    res = run_on_cores(in_maps).results

    out = np.concatenate(
        [res[c]["y"].astype(np.float32) for c in range(NCORES)], axis=0)
    return np.ascontiguousarray(out).reshape(B, S, O)


# revision 25
# speedup vs baseline: 1.9931x; 1.0824x over previous
"""Quantized (4-bit) LoRA linear for Trainium2, SPMD over 8 NeuronCores.

Math:  y[t,o] = sum_i x[t,i]*W[o,i] + bias[o] + 2.0 * sum_r (x@A^T)[t,r]*B[o,r]
where  W[o,i] = (nib[o,i] - zero[i]) * scale[i],  nib = unpacked 4-bit ints.

Strategy (fp8 DoubleRow): with xsF[t,i] = x[t,i]*scale[i]*F (F=128 lifts
values out of the fp8 subnormal range), split xsF = hi + lo into two
fp8e4m3 operands (error-feedback split, quantization error ~1e-3).  Then

  F*y[t,o] = sum_i (hi+lo)[t,i]*nib[o,i]     (fp8 DoubleRow matmuls; nib in
                                              {0..15} is exact in fp8)
           + G[t,:] @ H[:,o]                 (small fp8 DoubleRow matmul)
           - c_h[t]                          (fused into the DVE eviction as
                                              a per-partition scalar)
  followed by *(1/F) and fp16 store (cast to fp32 on host).

  G rows (x 1/64): 0 = c_l = sum_i xsF*z_lo, 1-8 = F*(x@A^T), 9 = 1 (bias)
  H rows (x 64):   0 = -1,  1-8 = 2*B^T,     9 = F*bias
  c_h[t] = sum_i xsF*z_hi, extracted to token-partitions via a one-hot
  matmul (uf16-slice^T @ e9) into spare columns of the u PSUM bank.
  zero = z_hi + z_lo is an fp8 error-feedback split like xsF.

DoubleRow (both operands fp8e4m3, max 240!) contracts 2 k-tiles (256 rows)
per instruction at 0.5 cycles/row = 4x the fp16 matmul rate; hi+lo -> 2x.

Schedule: 8-way token split (1024 tokens/core).  Work unit = one PSUM strip
[128 tokens x 512 outs] accumulating all 4096 contraction rows (64 DoubleRow
insts) + 2 LoRA insts.  Phase A streams xs (8 MB) + the first o-strip column
of nib j-major with 6 strips + 2 u-chain banks in flight, pacing the PE
right behind the DMA.  Steady state runs strips chain-major (nib o-columns
of 512 stream 16 tiles each, prefetched one column ahead), with each strip's
LoRA/evict/store trailing one strip behind so PSUM bank reuse never stalls
the PE.  Eviction is one DVE op: (psum - c_h[t]) * (1/F) -> fp16.
"""

import numpy as np

B, S, I, O = 4, 2048, 4096, 4096
T = B * S            # 8192 tokens
NCORES = 8
TC = T // NCORES     # 1024 tokens per core
KP = I // 256        # 16 k-pairs (DoubleRow contracts 256 rows each)
OH = O // 2          # o-half width
NTT = TC // 128      # 8 token tiles per core
NS = OH // 512       # 4 psum strips per o-half
F = 128.0            # fp8 pre-scale (power of two, exact)

_CACHE = {}


def _build_program():
    import concourse.bacc as bacc
    import concourse.mybir as mybir
    import concourse.tile as tile

    fp16 = mybir.dt.float16
    fp32 = mybir.dt.float32
    fp8 = mybir.dt.float8e4
    DR = mybir.MatmulPerfMode.DoubleRow

    nc = bacc.Bacc("TRN2", target_bir_lowering=False, debug=False)
    xs8 = nc.dram_tensor("xs8", [KP, 128, 2, 2, TC], fp8, kind="ExternalInput")
    nib8 = nc.dram_tensor("nib8", [KP, 128, 2, O], fp8, kind="ExternalInput")
    aext8 = nc.dram_tensor("aext8", [128, KP + 1, 2, 16], fp8, kind="ExternalInput")
    e9 = nc.dram_tensor("e9", [16, 1], fp16, kind="ExternalInput")
    sel16 = nc.dram_tensor("sel16", [16, 128], fp16, kind="ExternalInput")
    nibh8 = nc.dram_tensor("nibh8", [128, 2, O], fp8, kind="ExternalInput")
    y = nc.dram_tensor("y", [TC, O], fp16, kind="ExternalOutput")

    with tile.TileContext(nc) as tc:
        with (
            tc.tile_pool(name="xs", bufs=1) as xs_pool,
            tc.tile_pool(name="nib", bufs=22) as nib_pool,
            tc.tile_pool(name="consts", bufs=1) as const_pool,
            tc.tile_pool(name="g", bufs=1) as g_pool,
            tc.tile_pool(name="out", bufs=3) as out_pool,
            tc.tile_pool(name="psum", bufs=8, space="PSUM") as psum_pool,
        ):
            uf16 = g_pool.tile([16, TC], fp16, tag="uf", name="uf16")
            e9_tile = const_pool.tile([16, 1], fp16, tag="e9")
            sel_tile = const_pool.tile([16, 128], fp16, tag="sel")
            ct32all = g_pool.tile([128, NTT], fp32, tag="ct", name="ct32all")
            xs_tiles = [None] * KP
            aext_all = const_pool.tile([128, KP + 1, 2, 16], fp8, tag="aext")
            nib_tiles = {}  # (j, half) -> tile
            nc.vector.memset(gt8[:, :, :], 0.0)
            nc.vector.memset(gt8[:, 0, :], 1.0)  # const row (bias)

            def load_nib(j, h):
                nt = nib_pool.tile([128, 2, OH], fp8, tag="nib",
                                   name=f"nib{h}_{j}")
                nc.sync.dma_start(nt[:], nib8[j][:, :, h * OH:(h + 1) * OH])
                nib_tiles[(j, h)] = nt

            def main_insts(j, h, tt, ps, first):
                """The 4 DoubleRow insts of k-pair j for chain (h,tt,s)."""
                for s in range(NS):
                    for hl in range(2):
                        for c in range(2):
                            nc.tensor.matmul(
                                ps[s][:, c * 256:(c + 1) * 256],
                                xs_tiles[j][:, :, hl,
                                            tt * 128:(tt + 1) * 128],
                                nib_tiles[(j, h)][:, :,
                                                  s * 512 + c * 256:
                                                  s * 512 + (c + 1) * 256],
                                start=(first and hl == 0 and c == 0),
                                stop=False, perf_mode=DR,
                            )

            def finish_tt(h, tt, ps):
                """LoRA/zero/bias matmul + eviction + store for chain group."""
                ot = out_pool.tile([128, OH], fp16, tag="out",
                                   name=f"ot{h}_{tt}")
                for s in range(NS):
                    for c in range(2):
                        off = h * OH + s * 512 + c * 256
                        nc.tensor.matmul(
                            ps[s][:, c * 256:(c + 1) * 256],
                            gt8[:, :, tt * 128:(tt + 1) * 128],
                            h_tile[:, :, off:off + 256],
                            start=False, stop=(c == 1), perf_mode=DR,
                        )
                for s in range(NS):
                    nc.vector.tensor_scalar(
                        ot[:, s * 512:(s + 1) * 512], ps[s][:, :],
                        ct32_tiles[tt][:, :], 1.0 / F,
                        op0=mybir.AluOpType.subtract,
                        op1=mybir.AluOpType.mult)
                nc.scalar.dma_start(
                    y[tt * 128:(tt + 1) * 128, h * OH:(h + 1) * OH], ot[:])

            def new_ps(h, tt):
                return [psum_pool.tile([128, 512], fp32, tag="mm",
                                       name=f"ps{h}_{tt}_{s}")
                        for s in range(NS)]

            # ---- group 0 (half 0, tt 0 + first half of tt 1) + u chains ----
            # j-major emission right behind the streaming DMAs; 8 PSUM banks
            # all open (2 u + 4 tt0 + 2 tt1) to maximize PE fill while the
            # 16 MB of phase-A DMA (xs + nib half 0) streams in.
            up = [psum_pool.tile([16, 512], fp32, tag="mm", name=f"up{uc}")
                  for uc in range(2)]
            ps_prev = new_ps(0, 0)
            ps_tt1 = new_ps(0, 1)  # s0/s1 filled in group 0, s2/s3 in group 1
            for j in range(KP):
                xt = xs_pool.tile([128, 2, 2, TC], fp8, tag=f"xs{j}",
                                  name=f"xs{j}")
                nc.sync.dma_start(xt[:], xs8[j])
                xs_tiles[j] = xt
                if j == 0:
                    nc.sync.dma_start(aext_all[:], aext8[:, :, :, :])
                    nc.sync.dma_start(e9_tile[:], e9[:, :])
                    nc.sync.dma_start(sel_tile[:], sel16[:, :])
                if j == 1:
                    load_nibh(0)
                load_nib(j, 0)
                for uc in range(2):
                    for hl in range(2):
                        for c in range(2):
                            nc.tensor.matmul(
                                up[uc][:, c * 256:(c + 1) * 256],
                                aext_all[:, j, :, :],
                                xs_tiles[j][:, :, hl,
                                            uc * 512 + c * 256:
                                            uc * 512 + (c + 1) * 256],
                                start=(j == 0 and hl == 0 and c == 0),
                                stop=(j == KP - 1 and hl == 1 and c == 1),
                                perf_mode=DR,
                            )
                main_insts(j, 0, 0, ps_prev, first=(j == 0))
                for s in range(2):
                    for hl in range(2):
                        for c in range(2):
                            nc.tensor.matmul(
                                ps_tt1[s][:, c * 256:(c + 1) * 256],
                                xs_tiles[j][:, :, hl, 128:256],
                                nib_tiles[(j, 0)][:, :,
                                                  s * 512 + c * 256:
                                                  s * 512 + (c + 1) * 256],
                                start=(j == 0 and hl == 0 and c == 0),
                                stop=False, perf_mode=DR,
                            )
            for uc in range(2):
                nc.vector.tensor_copy(uf16[:, uc * 512:(uc + 1) * 512],
                                      up[uc][:, :])
            for tt in range(NTT):
                nc.scalar.dma_start_transpose(
                    ct_tiles[tt][:, :], uf16[:, tt * 128:(tt + 1) * 128])
            for tt in range(NTT):
                nc.gpsimd.tensor_copy(ct32_tiles[tt][:, :],
                                      ct_tiles[tt][:, 9:10])
            # early prefetch of half-1 nib into the 6 spare pool slots
            for j in range(6):
                load_nib(j, 1)
            # group 1: finish tt1 (strips 2,3 only)
            for j in range(KP):
                for s in (2, 3):
                    for hl in range(2):
                        for c in range(2):
                            nc.tensor.matmul(
                                ps_tt1[s][:, c * 256:(c + 1) * 256],
                                xs_tiles[j][:, :, hl, 128:256],
                                nib_tiles[(j, 0)][:, :,
                                                  s * 512 + c * 256:
                                                  s * 512 + (c + 1) * 256],
                                start=(j == 0 and hl == 0 and c == 0),
                                stop=False, perf_mode=DR,
                            )
            finish_tt(0, 0, ps_prev)
            ps_prev, tt_prev = ps_tt1, 1

            # ---- remaining groups, h/evict work staggered one group back ---
            for h in range(2):
                for tt in range(NTT):
                    if h == 0 and tt <= 1:
                        continue
                    if h == 1 and tt == NTT - 1:
                        break  # last group handled strip-major below
                    ps = new_ps(h, tt)
                    for j in range(KP):
                        if h == 1 and tt == 0 and j >= 6:
                            load_nib(j, 1)
                        main_insts(j, h, tt, ps, first=(j == 0))
                    finish_tt(h if tt > 0 else 0, tt_prev, ps_prev)
                    ps_prev, tt_prev = ps, tt

            # ---- last group (h=1, tt=7): strip-major so the tail drains
            # strip-by-strip instead of all-at-once after the final matmul.
            h, tt = 1, NTT - 1
            ps = new_ps(h, tt)
            ot_last = out_pool.tile([128, OH], fp16, tag="out", name="ot_last")
            for s in range(NS):
                for j in range(KP):
                    for hl in range(2):
                        for c in range(2):
                            nc.tensor.matmul(
                                ps[s][:, c * 256:(c + 1) * 256],
                                xs_tiles[j][:, :, hl,
                                            tt * 128:(tt + 1) * 128],
                                nib_tiles[(j, h)][:, :,
                                                  s * 512 + c * 256:
                                                  s * 512 + (c + 1) * 256],
                                start=(j == 0 and hl == 0 and c == 0),
                                stop=False, perf_mode=DR,
                            )
                if s == 0:
                    finish_tt(1, tt_prev, ps_prev)
                for c in range(2):
                    off = h * OH + s * 512 + c * 256
                    nc.tensor.matmul(
                        ps[s][:, c * 256:(c + 1) * 256],
                        gt8[:, :, tt * 128:(tt + 1) * 128],
                        h_tile[:, :, off:off + 256],
                        start=False, stop=(c == 1), perf_mode=DR,
                    )
                nc.vector.tensor_scalar(
                    ot_last[:, s * 512:(s + 1) * 512], ps[s][:, :],
                    ct32_tiles[tt][:, :], 1.0 / F,
                    op0=mybir.AluOpType.subtract, op1=mybir.AluOpType.mult)
                nc.scalar.dma_start(
                    y[tt * 128:(tt + 1) * 128,
                      h * OH + s * 512:h * OH + (s + 1) * 512],
                    ot_last[:, s * 512:(s + 1) * 512])
    nc.compile()
    return nc


def _prep_inputs(x, weight_quant, scale, zero, lora_A, lora_B, bias):
    """Host-side layout prep + sharding. Returns in_maps for 8 cores."""
    import ml_dtypes

    e4 = ml_dtypes.float8_e4m3   # device fp8e4: e4m3 WITH inf, max 240

    # xsF = x*scale*F, split hi/lo fp8, laid out [KP, 128, pair, hi/lo, T]
    xsF = (x.reshape(T, I).astype(np.float32) * (scale[None, :] * F))
    hi = xsF.astype(e4)
    lo = (xsF - hi.astype(np.float32)).astype(e4)

    def kshuf(arr):  # [I, ...] -> [KP, 128, 2, ...]
        return np.ascontiguousarray(
            arr.reshape(KP, 2, 128, *arr.shape[1:]).swapaxes(1, 2))

    xs8 = np.empty((KP, 128, 2, 2, T), e4)
    xs8[:, :, :, 0, :] = kshuf(np.ascontiguousarray(hi.T))
    xs8[:, :, :, 1, :] = kshuf(np.ascontiguousarray(lo.T))
    # rows k=4064..4073 (partitions 96-105, pair1 of the last k-tile) host
    # the planted gt rows in their lo slot: drop their lo-residuals, bake
    # the constant bias row (gt row 9 = 1.0) statically
    xs8[KP - 1, 96:105, 1, 1, :] = e4(0.0)
    xs8[KP - 1, 105, 1, 1, :] = e4(1.0)

    wq = weight_quant.astype(np.uint8)           # low byte only is populated
    nib = np.empty((O, I), np.uint8)
    nib[:, 0::2] = wq & 15
    nib[:, 1::2] = wq >> 4
    nib8 = kshuf(np.ascontiguousarray(nib.T).astype(e4))   # [KP,128,2,O]

    # u-matmul columns: 0 = z_lo residual, 1-8 = A/scale, 9 = z_hi
    aext = np.zeros((I, 16), np.float32)
    z_h = zero.astype(e4)
    z_l = (zero - z_h.astype(np.float32)).astype(e4)
    aext[:, 0] = z_l.astype(np.float32)
    aext[:, 1:9] = (lora_A.astype(np.float32) / scale[None, :]).T
    aext[:, 9] = z_h.astype(np.float32)
    aext8 = kshuf(aext.astype(e4)).transpose(1, 0, 2, 3)   # [128,KP,2,16]
    # slot KP: copy of slot KP-1 with the planted-row partitions zeroed in
    # pair 1 -- used by the (j15, lo) u-instruction so the planted gt rows
    # and the static const row contribute nothing to u/c
    aextlo = aext8[:, KP - 1:KP].copy()
    aextlo[96:106, 0, 1, :] = e4(0.0)
    aext8 = np.ascontiguousarray(
        np.concatenate([aext8, aextlo], axis=1))           # [128,KP+1,2,16]

    # H rows matching the planted gt rows [c_l/64, u0..7/64, 1.0]; they
    # ride in the hybrid last nib tile at (partitions 96-105, pair 1)
    hrows = np.zeros((10, O), np.float32)
    hrows[0] = -64.0
    hrows[1:9] = 128.0 * lora_B.T          # 64 * 2 * B^T
    hrows[9] = F * bias                    # const row carries the bias
    assert np.abs(hrows).max() < 240.0
    nibh8 = nib8[KP - 1].copy()            # [128, 2, O]
    nibh8[96:106, 1, :] = hrows.astype(e4)
    nibh8 = np.ascontiguousarray(nibh8)

    e9v = np.zeros((16, 1), np.float16)
    e9v[9, 0] = 1.0
    sel = np.zeros((16, 128), np.float16)
    for r in range(9):
        sel[r, 96 + r] = 1.0 / 64

    in_maps = []
    for c in range(NCORES):
        in_maps.append({
            "xs8": np.ascontiguousarray(xs8[..., c * TC:(c + 1) * TC]),
            "nib8": nib8,
            "aext8": aext8,
            "nibh8": nibh8,
            "e9": e9v,
            "sel16": sel,
        })
    return in_maps


def run_on_cores(in_maps, trace=False):
    from concourse.bass_utils import run_bass_kernel_spmd

    if "nc" not in _CACHE:
        _CACHE["nc"] = _build_program()
    return run_bass_kernel_spmd(
        _CACHE["nc"], in_maps, list(range(NCORES)), trace=trace
    )


def kernel(x, weight_quant, scale, zero, lora_A, lora_B, bias):
    x = np.asarray(x)
    weight_quant = np.asarray(weight_quant)
    scale = np.asarray(scale, np.float32)
    zero = np.asarray(zero, np.float32)
    lora_A = np.asarray(lora_A, np.float32)
    lora_B = np.asarray(lora_B, np.float32)
    bias = np.asarray(bias, np.float32)

    in_maps = _prep_inputs(x, weight_quant, scale, zero, lora_A, lora_B, bias)
    res = run_on_cores(in_maps).results

    out = np.concatenate(
        [res[c]["y"].astype(np.float32) for c in range(NCORES)], axis=0)
    return np.ascontiguousarray(out).reshape(B, S, O)
